# revision 2
# baseline (speedup 1.0000x reference)
"""Trainium2 Bass kernel v4 for nn_Cal_Div_Loss (conv-pyramid L1 loss).

HW computes, per sample: sd = sum(x-y), sa0 = sum|x-y|, and the level-1
pyramid d1 = edgesum(x-y) [255x255] (shipped to HBM in fp8). The host
finishes the tiny deep levels (d2..d4, 16x fewer elements each) and the
cross-batch sign/mean logic.

Per core (8 samples, 16 MiB input): DMA-bound at ~435 GB/s (~39 us
stream). v4 layout:
  - x loads ride the sync (SP) HWDGE ring, y loads the scalar (ACT)
    HWDGE ring -> both rings start at t=0 and share the 16 SDMA
    engines at packet granularity (aggregate line rate).
  - bt0 rides gpsimd SWDGE so neither HWDGE ring is delayed.
  - d1 evacs accumulate into two SBUF-resident fp8 tensors; ONE store
    per tensor at the end (2040B/partition descriptors, line rate)
    instead of per-block 510B-descriptor stores (which drained at
    ~30 GB/s for ~19 us in v3).
  - |d| reduction (sa0) runs on DVE as a second STT (max(-d, d) with
    accum) so the ACT queue only does PSUM evacs and nothing gates the
    final sample's chain.
Row r of an image maps to partition r//4, slot r%4 ("s (q g) c") so each
input DMA descriptor is one contiguous 8 KiB run.
"""

import sys

if "/opt/trn_rl_repo" not in sys.path:
    sys.path.insert(0, "/opt/trn_rl_repo")

import numpy as np

B = 64
NCORES = 8
S = B // NCORES      # samples per core
P = 128
N0, N1 = 512, 255
G0 = 4
LAYER_NUM = 4

BLOCKS = [(0, 1), (1, 2), (3, 2), (5, 2), (7, 1)]
STATS_COLS = 32      # [0:16] sd parity-halves (2/sample), [16:24] sa0

_CACHE = {}


def _banded_bt0():
    """R^T for window-3 stride-2 row sum, [512, 255] bf16."""
    import ml_dtypes

    r = np.zeros((N1, N0), dtype=np.float32)
    for i in range(N1):
        r[i, 2 * i : 2 * i + 3] = 1.0
    return np.ascontiguousarray(r.T).astype(ml_dtypes.bfloat16)


def _build_nc():
    from contextlib import ExitStack

    import concourse.bacc as bacc
    import concourse.mybir as mybir
    import concourse.tile as tile

    f32 = mybir.dt.float32
    bf16 = mybir.dt.bfloat16
    f8 = mybir.dt.float8e4
    ADD = mybir.AluOpType.add
    SUB = mybir.AluOpType.subtract
    MULT = mybir.AluOpType.mult
    MAX = mybir.AluOpType.max

    nc = bacc.Bacc("TRN2", target_bir_lowering=False, debug=False)
    xs = nc.dram_tensor("xs", [S, 512, 512], f32, kind="ExternalInput").ap()
    ys = nc.dram_tensor("ys", [S, 512, 512], f32, kind="ExternalInput").ap()
    bt0 = nc.dram_tensor("bt0", [512, N1], bf16, kind="ExternalInput").ap()
    d1a_out = nc.dram_tensor("d1a", [P, S, N1], f8, kind="ExternalOutput").ap()
    d1b_out = nc.dram_tensor("d1b", [127, S, N1], f8, kind="ExternalOutput").ap()
    stats_out = nc.dram_tensor(
        "stats", [P, STATS_COLS], f32, kind="ExternalOutput"
    ).ap()

    with tile.TileContext(nc, pool_alloc_mode="queue") as tc, ExitStack() as ctx:
        singles = ctx.enter_context(tc.tile_pool(name="singles", bufs=1))
        dpool = ctx.enter_context(tc.tile_pool(name="d", bufs=2))
        upool = ctx.enter_context(tc.tile_pool(name="u", bufs=3))
        vpool = ctx.enter_context(tc.tile_pool(name="v", bufs=2))
        apool = ctx.enter_context(tc.tile_pool(name="a", bufs=2))
        pa = ctx.enter_context(tc.tile_pool(name="pa", bufs=2, space="PSUM"))
        pb = ctx.enter_context(tc.tile_pool(name="pb", bufs=2, space="PSUM"))

        bt0_sb = singles.tile([P, G0, N1], bf16)
        nc.gpsimd.dma_start(out=bt0_sb, in_=bt0.rearrange("(q g) m -> q g m", g=G0))
        stats = singles.tile([P, STATS_COLS], f32)
        nc.vector.memset(stats, 0.0)

        # SBUF-resident fp8 d1 accumulation targets (stored once at end)
        d1a_sb = singles.tile([P, S, N1], f8)
        d1b_sb = singles.tile([127, S, N1], f8)

        # ---- all input DMAs issued up front: x on the sync ring,
        # y on the scalar ring; the two rings drain in parallel
        xyt = {}
        last_s0 = BLOCKS[-1][0]
        for s0, blk in BLOCKS:
            xt = singles.tile([P, blk, G0, N0], f32, name=f"xt{s0}")
            yt = singles.tile([P, blk, G0, N0], f32, name=f"yt{s0}")
            if s0 == last_s0 and blk == 1:
                # split the final sample into g-halves so its compute
                # chain starts before the whole sample has landed
                for h in range(2):
                    nc.sync.dma_start(
                        out=xt[:, 0, 2 * h : 2 * h + 2],
                        in_=xs[s0].rearrange(
                            "(q g) c -> q g c", g=G0
                        )[:, 2 * h : 2 * h + 2],
                    )
                    nc.scalar.dma_start(
                        out=yt[:, 0, 2 * h : 2 * h + 2],
                        in_=ys[s0].rearrange(
                            "(q g) c -> q g c", g=G0
                        )[:, 2 * h : 2 * h + 2],
                    )
            else:
                nc.sync.dma_start(
                    out=xt,
                    in_=xs[s0 : s0 + blk].rearrange("s (q g) c -> q s g c", g=G0),
                )
                nc.scalar.dma_start(
                    out=yt,
                    in_=ys[s0 : s0 + blk].rearrange("s (q g) c -> q s g c", g=G0),
                )
            xyt[s0] = (xt, yt)

        for bi, (s0, blk) in enumerate(BLOCKS):
            xt, yt = xyt[s0]

            # ---- level 0: subtract (parity planar) + colsum + stats ----
            dt = dpool.tile([P, 2, G0, 2, 256], bf16, tag="dt")
            v0 = vpool.tile([P, G0, 2, N1], bf16, tag="v0")
            for s in range(blk):
                j = s0 + s
                if s0 == last_s0 and blk == 1:
                    # per-g-half pipeline for the final sample
                    for h in range(2):
                        gs = slice(2 * h, 2 * h + 2)
                        sdc = (2 * j) if h == 0 else 24
                        for par in range(2):
                            nc.vector.scalar_tensor_tensor(
                                out=dt[:, s, gs, par, :],
                                in0=xt[:, s, gs, par : par + 511 : 2],
                                scalar=0.0,
                                in1=yt[:, s, gs, par : par + 511 : 2],
                                op0=ADD,
                                op1=SUB,
                                accum_out=stats[:, sdc + par : sdc + par + 1],
                            )
                        uth = upool.tile([P, 2, 256], bf16, tag="uth")
                        nc.vector.tensor_add(
                            out=uth,
                            in0=dt[:, s, gs, 0, :],
                            in1=dt[:, s, gs, 1, :],
                        )
                        nc.vector.tensor_add(
                            out=v0[:, gs, s, :],
                            in0=uth[:, :, 0:N1],
                            in1=dt[:, s, gs, 0, 1:256],
                        )
                        ath = apool.tile([P, 2, 2, 256], bf16, tag="ath")
                        nc.vector.scalar_tensor_tensor(
                            out=ath,
                            in0=dt[:, s, gs],
                            scalar=-1.0,
                            in1=dt[:, s, gs],
                            op0=MULT,
                            op1=MAX,
                            accum_out=stats[
                                :, (16 + j if h == 0 else 26) : (17 + j if h == 0 else 27)
                            ],
                        )
                    continue
                for par in range(2):
                    nc.vector.scalar_tensor_tensor(
                        out=dt[:, s, :, par, :],
                        in0=xt[:, s, :, par : par + 511 : 2],
                        scalar=0.0,
                        in1=yt[:, s, :, par : par + 511 : 2],
                        op0=ADD,
                        op1=SUB,
                        accum_out=stats[:, 2 * j + par : 2 * j + par + 1],
                    )
                ut = upool.tile([P, G0, 256], bf16, tag="ut")
                nc.vector.tensor_add(
                    out=ut, in0=dt[:, s, :, 0, :], in1=dt[:, s, :, 1, :]
                )
                nc.vector.tensor_add(
                    out=v0[:, :, s, :],
                    in0=ut[:, :, 0:N1],
                    in1=dt[:, s, :, 0, 1:256],
                )
                at = apool.tile([P, G0, 2, 256], bf16, tag="at")
                nc.vector.scalar_tensor_tensor(
                    out=at,
                    in0=dt[:, s],
                    scalar=-1.0,
                    in1=dt[:, s],
                    op0=MULT,
                    op1=MAX,
                    accum_out=stats[:, 16 + j : 17 + j],
                )

            # ---- banded row-sum matmuls (block fused, n=blk*255) ----
            # row r = 4q+g: m<128 touches rows<259 (q<65); m>=128 rows>=256
            w0 = pa.tile([P, 2, N1], f32, tag="pa")
            w1 = pb.tile([127, 2, N1], f32, tag="pb")
            for g in range(G0):
                nc.tensor.matmul(
                    w0[:, 0:blk],
                    bt0_sb[0:65, g, 0:128],
                    v0[0:65, g, 0:blk],
                    start=(g == 0),
                    stop=(g == G0 - 1),
                )
            for g in range(G0):
                nc.tensor.matmul(
                    w1[:, 0:blk],
                    bt0_sb[64:P, g, 128:N1],
                    v0[64:P, g, 0:blk],
                    start=(g == 0),
                    stop=(g == G0 - 1),
                )

            # ---- evacuate d1 into the SBUF fp8 accumulation tensors ----
            nc.scalar.copy(out=d1a_sb[:, s0 : s0 + blk, :], in_=w0[:, 0:blk])
            nc.scalar.copy(out=d1b_sb[:, s0 : s0 + blk, :], in_=w1[:, 0:blk])

        # ---- single end-of-kernel stores (large contiguous descriptors) ----
        nc.sync.dma_start(out=stats_out, in_=stats)
        nc.sync.dma_start(out=d1a_out, in_=d1a_sb)
        nc.scalar.dma_start(out=d1b_out, in_=d1b_sb)

    nc.finalize()
    return nc


def _get_nc():
    if "nc" not in _CACHE:
        _CACHE["nc"] = _build_nc()
    return _CACHE["nc"]


def _run_on_hw(x, y, trace=False):
    from concourse.bass_utils import run_bass_kernel_spmd

    nc = _get_nc()
    bt0 = _banded_bt0()
    in_maps = [
        {
            "xs": np.ascontiguousarray(x[c * S : (c + 1) * S]),
            "ys": np.ascontiguousarray(y[c * S : (c + 1) * S]),
            "bt0": bt0,
        }
        for c in range(NCORES)
    ]
    res = run_bass_kernel_spmd(
        nc, in_maps, core_ids=list(range(NCORES)), trace=trace
    )
    _CACHE["last_results"] = res
    return res.results


def _edgesum_np(a):
    """conv 3x3, stride 2, VALID, all-ones kernel. a: [B, H, W]."""
    r = a[:, 0:-2:2] + a[:, 1:-1:2] + a[:, 2::2]
    return r[:, :, 0:-2:2] + r[:, :, 1:-1:2] + r[:, :, 2::2]


def kernel(x, y, alpha, _trace=False):
    x = np.ascontiguousarray(np.asarray(x, dtype=np.float32).reshape(B, 512, 512))
    y = np.ascontiguousarray(np.asarray(y, dtype=np.float32).reshape(B, 512, 512))
    alpha = np.asarray(alpha, dtype=np.float32)

    results = _run_on_hw(x, y, trace=_trace)

    sd = np.empty(B, np.float64)
    sa0 = np.empty(B, np.float64)
    d1 = np.empty((B, N1, N1), np.float32)
    for c in range(NCORES):
        r = results[c]
        st = np.asarray(r["stats"]).astype(np.float64).sum(axis=0)  # [STATS_COLS]
        d1a = np.asarray(r["d1a"]).astype(np.float32)  # [128, S, 255]
        d1b = np.asarray(r["d1b"]).astype(np.float32)  # [127, S, 255]
        for j in range(S):
            b = c * S + j
            sd[b] = st[2 * j] + st[2 * j + 1]
            sa0[b] = st[16 + j]
            if j == S - 1:
                sd[b] += st[24] + st[25]
                sa0[b] += st[26]
            d1[b, 0:128, :] = d1a[:, j, :]
            d1[b, 128:N1, :] = d1b[:, j, :]

    # host: deep pyramid levels from d1
    l1 = np.empty((B, LAYER_NUM + 1), np.float64)
    l1[:, 0] = sa0 / float(N0 * N0)
    dl = d1
    for k in range(1, LAYER_NUM + 1):
        l1[:, k] = np.abs(dl.astype(np.float64)).mean(axis=(1, 2))
        if k < LAYER_NUM:
            dl = _edgesum_np(dl)

    last = np.abs(sd) * float(LAYER_NUM + 1)

    # faithful 'fuhao' replication (matches reference.py exactly)
    k_layer = (alpha * np.float32(LAYER_NUM + 2)).astype(np.int32)
    trig = k_layer <= LAYER_NUM
    triggered_before = np.concatenate(
        [np.zeros(1, bool), np.cumsum(trig)[:-1] > 0]
    )
    i_idx = np.arange(LAYER_NUM + 1)
    sign = np.where(
        triggered_before[:, None] | (i_idx[None, :] >= k_layer[:, None]),
        1.0,
        -1.0,
    )

    loss_tensor = np.concatenate([l1 * sign, last[:, None]], axis=1)
    return np.float32(loss_tensor.mean())
